# revision 1
# baseline (speedup 1.0000x reference)
"""Trainium2 Bass kernel for the Enigma-style CopyMemoryModel.

Math (validated vs reference):
  - The lax.scan carries nothing -> every timestep t is independent.
  - t < 128 and d = 1024  =>  rotors 1,2 have pos = 0 (no roll); only rotor 0
    rolls by t.
  - Host-folded weights:  Wpre = P@Wi (+ bias row),  W12 = rotW1@rotW2,
    Srefl = R+R.T,  Wpost = P@Wo.T.  bo added on host after gather.
  - roll(h, +-t) along D is expressed per 128-row block as two shifted-diagonal
    128x128 matmuls (diag + off-diag wrap), batched over all 8 blocks per
    t-group with strided rhs access patterns.
  - Layout on chip: activations stored transposed, hT[128 part, 8 blocks x 1024
    tokens] per core; every stage is out_block[jt] = sum_kt W[kt,jt].T @ h[kt].

Sharding: time-sharded; core c handles t in [c*16, (c+1)*16), all 64 batch
samples -> 1024 tokens per core, token column = g*64 + b.
"""
import numpy as np

B, S, DIN, D, DOUT = 64, 128, 64, 1024, 64
NCORES = 8
TLOC = S // NCORES          # 16 timesteps per core
NTOK = B * TLOC             # 1024 tokens per core
NB = D // 128               # 8 row blocks
NCH = NTOK // 512           # 2 column chunks of 512
HB = 512                    # half of D (rev-block split)
NBH = HB // 128             # 4 blocks per half

_compiled = {}


def _build():
    import concourse.bacc as bacc
    import concourse.mybir as mybir
    from concourse.tile import TileContext

    f32 = mybir.dt.float32
    f32r = mybir.dt.float32r
    ACT_TANH = mybir.ActivationFunctionType.Tanh
    ACT_COPY = mybir.ActivationFunctionType.Copy

    nc = bacc.Bacc(None, target_bir_lowering=False, debug=True)

    xt_d = nc.dram_tensor("xt", [DIN + 1, NTOK], f32r, kind="ExternalInput")
    pf_d = nc.dram_tensor("permf", [128, TLOC * 256], f32r, kind="ExternalInput")
    pb_d = nc.dram_tensor("permb", [128, TLOC * 256], f32r, kind="ExternalInput")
    wpre_d = nc.dram_tensor("wpre", [DIN + 1, D], f32r, kind="ExternalInput")
    wbig_d = nc.dram_tensor("wbig", [5, 128, NB * D], f32r, kind="ExternalInput")
    wf_d = nc.dram_tensor("wf", [3, 128, NBH * HB], f32r, kind="ExternalInput")
    wg_d = nc.dram_tensor("wg", [3, 128, NBH * HB], f32r, kind="ExternalInput")
    wpost_d = nc.dram_tensor("wpost", [128, NB * DOUT], f32r, kind="ExternalInput")
    out_d = nc.dram_tensor("out", [DOUT, NTOK], f32, kind="ExternalOutput")

    with TileContext(nc) as tc:
        with (
            tc.tile_pool(name="hbuf", bufs=1) as hpool,
            tc.tile_pool(name="wpool", bufs=2) as wpool,
            tc.tile_pool(name="fgpool", bufs=3) as fgpool,
            tc.tile_pool(name="ppool", bufs=1) as ppool,
            tc.tile_pool(name="cpool", bufs=1) as cpool,
            tc.tile_pool(name="tpool", bufs=2) as tpool,
            tc.tile_pool(name="ps1", bufs=6, space="PSUM") as ps1,
            tc.tile_pool(name="psw", bufs=1, space="PSUM") as psw,
        ):
            hA = hpool.tile([128, NB * NTOK], f32r)
            hB = hpool.tile([128, NB * NTOK], f32r)

            xt = cpool.tile([DIN + 1, NTOK], f32r)
            wpre = cpool.tile([DIN + 1, D], f32r)
            wpost = cpool.tile([128, NB * DOUT], f32r)
            outsb = cpool.tile([DOUT, NTOK], f32)

            nc.sync.dma_start(xt[:], xt_d[:])
            nc.sync.dma_start(wpre[:], wpre_d[:])
            nc.sync.dma_start(wpost[:], wpost_d[:])
            # first roll stage's perms: queue the DMA as early as possible
            pall_f1 = ppool.tile([128, TLOC * 256], f32r, tag="pall")
            for q in range(4):
                nc.sync.dma_start(pall_f1[:, q * 1024:(q + 1) * 1024],
                                  pf_d[:, q * 1024:(q + 1) * 1024])

            # HAM warmup: dependency-free matmuls on a memset tile so the PE
            # clock un-throttles (and the PE isn't idle) while input DMAs land.
            junk = cpool.tile([128, 512], f32)
            nc.gpsimd.memset(junk[:], 0.0)

            def junk_mm(n):
                for r in range(n):
                    wps = psw.tile([128, 512], f32, tag="wps")
                    nc.tensor.matmul(wps[:, 0:128], junk[:, 0:128],
                                     junk[:, 0:128], start=True, stop=True)

            with nc.named_scope("warmup"):
                junk_mm(24)

            def big_stage(src, dst, slot):
                w = wpool.tile([128, NB * D], f32r)
                for q in range(4):
                    nc.sync.dma_start(w[:, q * 2048:(q + 1) * 2048],
                                      wbig_d[slot, :, q * 2048:(q + 1) * 2048])
                for ch in range(NCH):
                    for jt in range(NB):
                        ps = ps1.tile([128, 512], f32)
                        for kt in range(NB):
                            nc.tensor.matmul(
                                ps[:],
                                w[:, kt * D + jt * 128:kt * D + (jt + 1) * 128],
                                src[:, kt * NTOK + ch * 512:kt * NTOK + (ch + 1) * 512],
                                start=(kt == 0), stop=(kt == NB - 1),
                            )
                        nc.scalar.activation(
                            dst[:, jt * NTOK + ch * 512:jt * NTOK + (ch + 1) * 512],
                            ps[:], ACT_COPY)

            def roll_stage(src, dst, fwd, pall=None, fill=False):
                srcR = src[:].rearrange("p (n t) -> p n t", n=NB)
                dstR = dst[:].rearrange("p (n t) -> p n t", n=NB)
                if pall is None:
                    # all 16 perm pairs for this stage in one prefetchable DMA
                    pall = ppool.tile([128, TLOC * 256], f32r, tag="pall")
                    pd = (pf_d if fwd else pb_d)
                    for q in range(4):
                        nc.sync.dma_start(pall[:, q * 1024:(q + 1) * 1024],
                                          pd[:, q * 1024:(q + 1) * 1024])
                for g in range(TLOC):
                    gs, ge = g * B, (g + 1) * B
                    ps = ps1.tile([128, 512], f32)
                    U = pall[:, g * 256:g * 256 + 128]
                    L = pall[:, g * 256 + 128:(g + 1) * 256]
                    if fwd:
                        # out[jt] = U.T @ src[jt] + L.T @ src[(jt+1)%8]
                        nc.tensor.matmul(ps[:, 0:448], U, srcR[:, 0:7, gs:ge],
                                         start=True, stop=False)
                        nc.tensor.matmul(ps[:, 0:448], L, srcR[:, 1:8, gs:ge],
                                         start=False, stop=True)
                        nc.tensor.matmul(ps[:, 448:512], U, srcR[:, 7, gs:ge],
                                         start=True, stop=False)
                        nc.tensor.matmul(ps[:, 448:512], L, srcR[:, 0, gs:ge],
                                         start=False, stop=True)
                    else:
                        # out[jt] = U @ src[jt] + L @ src[(jt-1)%8]; lhsT in
                        # permb is already [U.T | L.T]
                        nc.tensor.matmul(ps[:, 64:512], U, srcR[:, 1:8, gs:ge],
                                         start=True, stop=False)
                        nc.tensor.matmul(ps[:, 64:512], L, srcR[:, 0:7, gs:ge],
                                         start=False, stop=True)
                        nc.tensor.matmul(ps[:, 0:64], U, srcR[:, 0, gs:ge],
                                         start=True, stop=False)
                        nc.tensor.matmul(ps[:, 0:64], L, srcR[:, 7, gs:ge],
                                         start=False, stop=True)
                    psR = ps[:].rearrange("p (n t) -> p n t", n=NB)
                    nc.vector.tensor_copy(dstR[:, :, gs:ge], psR)
                    if fill:
                        junk_mm(2)   # bridge HAM activity during gappy phase

            def coupling(buf, w_dram, w_idx, in_half, out_half):
                # buf[out_half] += tanh(W.T @ buf[in_half]), per 128-block
                fg = fgpool.tile([128, NBH * HB], f32r)
                nc.sync.dma_start(fg[:], w_dram[w_idx])
                for ch in range(NCH):
                    for jt in range(NBH):
                        ps = ps1.tile([128, 512], f32)
                        for kt in range(NBH):
                            nc.tensor.matmul(
                                ps[:],
                                fg[:, kt * HB + jt * 128:kt * HB + (jt + 1) * 128],
                                buf[:, (in_half * NBH + kt) * NTOK + ch * 512:
                                    (in_half * NBH + kt) * NTOK + (ch + 1) * 512],
                                start=(kt == 0), stop=(kt == NBH - 1),
                            )
                        tmp = tpool.tile([128, 512], f32r)
                        nc.scalar.activation(tmp[:], ps[:], ACT_TANH)
                        dsl = buf[:, (out_half * NBH + jt) * NTOK + ch * 512:
                                  (out_half * NBH + jt) * NTOK + (ch + 1) * 512]
                        nc.vector.tensor_add(dsl, dsl, tmp[:])

            def rev_block(buf, i):
                coupling(buf, wf_d, i, in_half=1, out_half=0)   # y1 = h1 + tanh(F.T h2)
                coupling(buf, wg_d, i, in_half=0, out_half=1)   # y2 = h2 + tanh(G.T y1)

            def scoped(name, fn, *args, **kw):
                with nc.named_scope(name):
                    fn(*args, **kw)

            # stage 1: input projection (K = DIN+1 with ones row for bias)
            for ch in range(NCH):
                for jt in range(NB):
                    ps = ps1.tile([128, 512], f32)
                    nc.tensor.matmul(ps[:], wpre[:, jt * 128:(jt + 1) * 128],
                                     xt[:, ch * 512:(ch + 1) * 512],
                                     start=True, stop=True)
                    nc.scalar.activation(
                        hA[:, jt * NTOK + ch * 512:jt * NTOK + (ch + 1) * 512],
                        ps[:], ACT_COPY)

            scoped("roll_f1", roll_stage, hA, hB, fwd=True, pall=pall_f1)    # u = roll(h, -t)
            scoped("W0", big_stage, hB, hA, 0)                 # @ rotW0
            scoped("roll_b1", roll_stage, hA, hB, fwd=False)   # roll back (+t)
            scoped("W12", big_stage, hB, hA, 1)                # @ W12
            for i in range(3):
                scoped(f"revf{i}", rev_block, hA, i)
            scoped("Srefl", big_stage, hA, hB, 2)              # @ Srefl
            for i in reversed(range(3)):
                scoped(f"revb{i}", rev_block, hB, i)
            scoped("W12T", big_stage, hB, hA, 3)               # @ W12.T
            scoped("roll_f2", roll_stage, hA, hB, fwd=True)
            scoped("W0T", big_stage, hB, hA, 4)                # @ rotW0.T
            scoped("roll_b2", roll_stage, hA, hB, fwd=False)

            # output projection: out[64, tok] = sum_kt wpost[kt].T @ hB[kt]
            with nc.named_scope("out"):
                for ch in range(NCH):
                    ps = ps1.tile([DOUT, 512], f32)
                    for kt in range(NB):
                        nc.tensor.matmul(
                            ps[:], wpost[:, kt * DOUT:(kt + 1) * DOUT],
                            hB[:, kt * NTOK + ch * 512:kt * NTOK + (ch + 1) * 512],
                            start=(kt == 0), stop=(kt == NB - 1))
                    nc.scalar.activation(outsb[:, ch * 512:(ch + 1) * 512], ps[:],
                                         ACT_COPY)
                    nc.sync.dma_start(out_d[:, ch * 512:(ch + 1) * 512],
                                      outsb[:, ch * 512:(ch + 1) * 512])

    nc.compile()
    return nc


def _host_weights(Wi, bi, P, rotW, F, G, R, Wo):
    """Fold weights on host into the exact SBUF layouts the kernel expects."""
    Wpre = P @ Wi                               # [D, DIN]
    bpre = P @ bi                               # [D]
    WpreA = np.concatenate([Wpre, bpre[:, None]], axis=1)      # [D, DIN+1]
    wpre = np.ascontiguousarray(WpreA.T, np.float32)           # [DIN+1, D]

    W12 = rotW[1] @ rotW[2]
    bigs = [rotW[0], W12, R + R.T, np.ascontiguousarray(W12.T),
            np.ascontiguousarray(rotW[0].T)]
    wbig = np.stack([w.reshape(NB, 128, D).transpose(1, 0, 2).reshape(128, NB * D)
                     for w in bigs]).astype(np.float32)        # [5, 128, 8*D]

    wf = np.stack([f.reshape(NBH, 128, HB).transpose(1, 0, 2).reshape(128, NBH * HB)
                   for f in F]).astype(np.float32)
    wg = np.stack([g.reshape(NBH, 128, HB).transpose(1, 0, 2).reshape(128, NBH * HB)
                   for g in G]).astype(np.float32)

    Wpost = P @ Wo.T                                           # [D, DOUT]
    wpost = Wpost.reshape(NB, 128, DOUT).transpose(1, 0, 2).reshape(128, NB * DOUT)
    wpost = np.ascontiguousarray(wpost, np.float32)
    return wpre, wbig, wf, wg, wpost


def _perms_for_core(c):
    pf = np.zeros((TLOC, 128, 256), np.float32)
    pb = np.zeros((TLOC, 128, 256), np.float32)
    for g in range(TLOC):
        t = c * TLOC + g
        U = np.eye(128, k=-t, dtype=np.float32)       # fwd diag lhsT
        L = np.eye(128, k=128 - t, dtype=np.float32)  # fwd off-diag lhsT
        pf[g, :, 0:128] = U
        pf[g, :, 128:256] = L
        pb[g, :, 0:128] = U.T                          # bwd diag lhsT
        pb[g, :, 128:256] = L.T
    # sbuf layout: [128 part, g*256 + f]
    pf = np.ascontiguousarray(pf.transpose(1, 0, 2).reshape(128, TLOC * 256))
    pb = np.ascontiguousarray(pb.transpose(1, 0, 2).reshape(128, TLOC * 256))
    return pf, pb


def kernel(x, Wi, bi, P, rotW, F, G, R, Wo, bo):
    x = np.asarray(x, np.float32)
    Wi, bi, P = (np.asarray(a, np.float32) for a in (Wi, bi, P))
    rotW, F, G = (np.asarray(a, np.float32) for a in (rotW, F, G))
    R, Wo, bo = (np.asarray(a, np.float32) for a in (R, Wo, bo))

    if "nc" not in _compiled:
        _compiled["nc"] = _build()
    nc = _compiled["nc"]

    wpre, wbig, wf, wg, wpost = _host_weights(Wi, bi, P, rotW, F, G, R, Wo)

    in_maps = []
    for c in range(NCORES):
        # xt[din, g*B + b] = x[b, c*TLOC + g, din]; ones row for the bias
        xs = x[:, c * TLOC:(c + 1) * TLOC, :]          # [B, TLOC, DIN]
        xT = xs.transpose(2, 1, 0).reshape(DIN, NTOK)  # [DIN, g*B+b]
        xT = np.concatenate([xT, np.ones((1, NTOK), np.float32)], axis=0)
        pf, pb = _perms_for_core(c)
        in_maps.append({
            "xt": np.ascontiguousarray(xT),
            "permf": pf, "permb": pb,
            "wpre": wpre, "wbig": wbig, "wf": wf, "wg": wg, "wpost": wpost,
        })

    from concourse.bass_utils import run_bass_kernel_spmd
    res = run_bass_kernel_spmd(nc, in_maps, list(range(NCORES)))
    _compiled["last_res"] = res

    out = np.empty((B, S, DOUT), np.float32)
    for c in range(NCORES):
        oT = res.results[c]["out"]                     # [DOUT, NTOK]
        out[:, c * TLOC:(c + 1) * TLOC, :] = \
            oT.reshape(DOUT, TLOC, B).transpose(2, 1, 0)
    out += bo.astype(np.float32)
    return out



# revision 5
# speedup vs baseline: 1.8253x; 1.8253x over previous
"""Trainium2 Bass kernel for the Enigma-style CopyMemoryModel (v2).

Math (validated vs reference):
  - The lax.scan carries nothing -> every timestep t is independent.
  - t < 128 and d = 1024  =>  rotors 1,2 have pos = 0; only rotor 0 rolls.
  - The entire pre-nonlinear chain ( @(P@Wi).T+bias, roll(-t), @rotW0,
    roll(+t), @rotW1@rotW2 ) is LINEAR per timestep and folds on the host
    into a single [65, 1024] matrix B_t per t.  Likewise the post-chain
    ( @W12.T, roll(-t), @rotW0.T, roll(+t), @P, @Wo.T ) folds into a single
    [1024, 64] matrix C_t.  On-chip work is then only:
        z = x~ @ B_t            (bf16 weights+inputs, fp32 accumulate)
        3 fwd rev-blocks, reflector (R+R.T), 3 bwd rev-blocks   (fp32)
        out = q @ C_t           (fp32)
    This removes 3 of the 5 big 1024x1024 stages and all 4 roll stages.
  - bo added on host after gather.

Layout on chip: activations transposed, h[128 part, 8 blocks x 1024
tokens]; every stage is out_block[jt] = sum_kt W[kt,jt].T @ h[kt].

Sharding: time-sharded; core c handles t in [c*16, (c+1)*16), all 64 batch
samples -> 1024 tokens per core, token column = g*64 + b.
"""
import numpy as np

B, S, DIN, D, DOUT = 64, 128, 64, 1024, 64
NCORES = 8
TLOC = S // NCORES          # 16 timesteps per core
NTOK = B * TLOC             # 1024 tokens per core
NB = D // 128               # 8 row blocks
NCH = NTOK // 512           # 2 column chunks of 512
HB = 512                    # half of D (rev-block split)
NBH = HB // 128             # 4 blocks per half
KIN = DIN + 1               # input dim + ones row for bias

_compiled = {}


def _build():
    import concourse.bacc as bacc
    import concourse.mybir as mybir
    from concourse.tile import TileContext

    f32 = mybir.dt.float32
    f32r = mybir.dt.float32r
    bf16 = mybir.dt.bfloat16
    ACT_TANH = mybir.ActivationFunctionType.Tanh
    ACT_COPY = mybir.ActivationFunctionType.Copy

    nc = bacc.Bacc(None, target_bir_lowering=False, debug=True)

    xt_d = nc.dram_tensor("xtb", [KIN, NTOK], bf16, kind="ExternalInput")
    wpb_d = nc.dram_tensor("wpb", [KIN, TLOC * D], bf16, kind="ExternalInput")
    wf_d = nc.dram_tensor("wf", [3, 128, NBH * HB], f32r, kind="ExternalInput")
    wg_d = nc.dram_tensor("wg", [3, 128, NBH * HB], f32r, kind="ExternalInput")
    wbigS_d = nc.dram_tensor("wbigS", [128, NB * D], f32r, kind="ExternalInput")
    wpc_d = nc.dram_tensor("wpc", [128, TLOC * NB * DOUT], f32r,
                           kind="ExternalInput")
    out_d = nc.dram_tensor("out", [DOUT, NTOK], f32, kind="ExternalOutput")

    with TileContext(nc) as tc:
        with (
            tc.tile_pool(name="hbuf", bufs=1) as hpool,
            tc.tile_pool(name="cpool", bufs=1) as cpool,
            tc.tile_pool(name="fgpool", bufs=2) as fgpool,
            tc.tile_pool(name="tpool", bufs=2) as tpool,
            tc.tile_pool(name="ps1", bufs=5, space="PSUM") as ps1,
            tc.tile_pool(name="psp", bufs=2, space="PSUM") as psp,
            tc.tile_pool(name="psw", bufs=1, space="PSUM") as psw,
        ):
            hA = hpool.tile([128, NB * NTOK], f32r)
            hB = hpool.tile([128, NB * NTOK], f32r)

            xtb = cpool.tile([KIN, NTOK], bf16)
            wpb = cpool.tile([KIN, TLOC * D], bf16)
            wbigS = cpool.tile([128, NB * D], f32r)
            wpc = cpool.tile([128, TLOC * NB * DOUT], f32r)
            outsb = cpool.tile([DOUT, NTOK], f32)

            # critical-path DMAs first: pre-projection inputs
            nc.sync.dma_start(xtb[:], xt_d[:])
            for q in range(4):
                nc.sync.dma_start(wpb[:, q * 4096:(q + 1) * 4096],
                                  wpb_d[:, q * 4096:(q + 1) * 4096])
            # first two couplings' weights early so they don't queue behind
            # the big reflector/post-projection weight DMAs
            fgF0 = fgpool.tile([128, NBH * HB], f32r, tag="fg")
            nc.sync.dma_start(fgF0[:], wf_d[0])
            fgG0 = fgpool.tile([128, NBH * HB], f32r, tag="fg")
            nc.sync.dma_start(fgG0[:], wg_d[0])
            for q in range(4):
                nc.sync.dma_start(wbigS[:, q * 2048:(q + 1) * 2048],
                                  wbigS_d[:, q * 2048:(q + 1) * 2048])
            for q in range(4):
                nc.sync.dma_start(wpc[:, q * 2048:(q + 1) * 2048],
                                  wpc_d[:, q * 2048:(q + 1) * 2048])

            # HAM warmup: dependency-free matmuls on a memset tile so the PE
            # clock un-throttles (and the PE isn't idle) while input DMAs land.
            junk = cpool.tile([128, 512], f32)
            nc.gpsimd.memset(junk[:], 0.0)

            def junk_mm(n):
                for r in range(n):
                    wps = psw.tile([128, 512], f32, tag="wps")
                    nc.tensor.matmul(wps[:, 0:128], junk[:, 0:128],
                                     junk[:, 0:128], start=True, stop=True)

            with nc.named_scope("warmup"):
                junk_mm(32)

            # pre-projection: z[g] = B_t.T @ x~[g]  (bf16 in, fp32 psum)
            with nc.named_scope("pre"):
                hAr = hA[:].rearrange("p (n t) -> p n t", n=NB)
                for g in range(TLOC):
                    ps = ps1.tile([128, 512], f32)
                    for jt in range(NB):
                        nc.tensor.matmul(
                            ps[:, jt * B:(jt + 1) * B],
                            wpb[:, (g * NB + jt) * 128:(g * NB + jt + 1) * 128],
                            xtb[:, g * B:(g + 1) * B],
                            start=True, stop=True)
                    psr = ps[:].rearrange("p (n t) -> p n t", n=NB)
                    nc.scalar.activation(hAr[:, :, g * B:(g + 1) * B], psr,
                                         ACT_COPY)

            def coupling(buf, w_dram, w_idx, in_half, out_half, fg=None):
                # buf[out_half] += tanh(W.T @ buf[in_half]), per 128-block
                if fg is None:
                    fg = fgpool.tile([128, NBH * HB], f32r, tag="fg")
                    nc.sync.dma_start(fg[:], w_dram[w_idx])
                for ch in range(NCH):
                    for jt in range(NBH):
                        ps = ps1.tile([128, 512], f32)
                        for kt in range(NBH):
                            nc.tensor.matmul(
                                ps[:],
                                fg[:, kt * HB + jt * 128:kt * HB + (jt + 1) * 128],
                                buf[:, (in_half * NBH + kt) * NTOK + ch * 512:
                                    (in_half * NBH + kt) * NTOK + (ch + 1) * 512],
                                start=(kt == 0), stop=(kt == NBH - 1),
                            )
                        tmp = tpool.tile([128, 512], f32r)
                        nc.scalar.activation(tmp[:], ps[:], ACT_TANH)
                        dsl = buf[:, (out_half * NBH + jt) * NTOK + ch * 512:
                                  (out_half * NBH + jt) * NTOK + (ch + 1) * 512]
                        nc.vector.tensor_add(dsl, dsl, tmp[:])

            def rev_block(buf, i, fgF=None, fgG=None):
                coupling(buf, wf_d, i, in_half=1, out_half=0, fg=fgF)
                coupling(buf, wg_d, i, in_half=0, out_half=1, fg=fgG)

            def scoped(name, fn, *args, **kw):
                with nc.named_scope(name):
                    fn(*args, **kw)

            scoped("revf0", rev_block, hA, 0, fgF=fgF0, fgG=fgG0)
            for i in range(1, 3):
                scoped(f"revf{i}", rev_block, hA, i)

            # reflector: hB[jt] = sum_kt S[kt,jt].T @ hA[kt]
            with nc.named_scope("srefl"):
                for ch in range(NCH):
                    for jt in range(NB):
                        ps = ps1.tile([128, 512], f32)
                        for kt in range(NB):
                            nc.tensor.matmul(
                                ps[:],
                                wbigS[:, kt * D + jt * 128:kt * D + (jt + 1) * 128],
                                hA[:, kt * NTOK + ch * 512:kt * NTOK + (ch + 1) * 512],
                                start=(kt == 0), stop=(kt == NB - 1),
                            )
                        nc.scalar.activation(
                            hB[:, jt * NTOK + ch * 512:jt * NTOK + (ch + 1) * 512],
                            ps[:], ACT_COPY)

            for i in reversed(range(3)):
                scoped(f"revb{i}", rev_block, hB, i)

            # post-projection: out[g] = sum_kt C_t[kt].T @ q[kt]
            with nc.named_scope("post"):
                for g in range(TLOC):
                    pb = psp.tile([DOUT, B], f32)
                    for kt in range(NB):
                        nc.tensor.matmul(
                            pb[:],
                            wpc[:, (g * NB + kt) * DOUT:(g * NB + kt + 1) * DOUT],
                            hB[:, kt * NTOK + g * B:kt * NTOK + (g + 1) * B],
                            start=(kt == 0), stop=(kt == NB - 1))
                    nc.scalar.activation(outsb[:, g * B:(g + 1) * B], pb[:],
                                         ACT_COPY)
                    if g == TLOC // 2 - 1:
                        nc.sync.dma_start(out_d[:, 0:512], outsb[:, 0:512])
                nc.sync.dma_start(out_d[:, 512:1024], outsb[:, 512:1024])

    nc.compile()
    return nc


def _host_weights(Wi, bi, P, rotW, F, G, R, Wo):
    """Fold weights on host into the exact SBUF layouts the kernel expects."""
    W0 = rotW[0]
    W12 = rotW[1] @ rotW[2]
    Wpre = P @ Wi                               # [D, DIN]
    bpre = P @ bi                               # [D]
    A = np.concatenate([Wpre, bpre[:, None]], axis=1).T        # [KIN, D]
    A = np.ascontiguousarray(A, np.float32)
    Wpost = (P @ Wo.T).astype(np.float32)                      # [D, DOUT]

    W0T = np.ascontiguousarray(W0.T)
    W12T = np.ascontiguousarray(W12.T)
    Ball = np.empty((S, KIN, D), np.float32)
    Call = np.empty((S, D, DOUT), np.float32)
    for t in range(S):
        Ball[t] = np.roll(np.roll(A, -t, axis=1) @ W0, t, axis=1) @ W12
        Call[t] = W12T @ np.roll(W0T @ np.roll(Wpost, -t, axis=0), t, axis=0)

    Srefl = R + R.T
    wbigS = Srefl.reshape(NB, 128, D).transpose(1, 0, 2).reshape(128, NB * D)
    wbigS = np.ascontiguousarray(wbigS, np.float32)

    wf = np.stack([f.reshape(NBH, 128, HB).transpose(1, 0, 2).reshape(128, NBH * HB)
                   for f in F]).astype(np.float32)
    wg = np.stack([g.reshape(NBH, 128, HB).transpose(1, 0, 2).reshape(128, NBH * HB)
                   for g in G]).astype(np.float32)
    return Ball, Call, wbigS, wf, wg


def kernel(x, Wi, bi, P, rotW, F, G, R, Wo, bo):
    import ml_dtypes
    bf16 = ml_dtypes.bfloat16

    x = np.asarray(x, np.float32)
    Wi, bi, P = (np.asarray(a, np.float32) for a in (Wi, bi, P))
    rotW, F, G = (np.asarray(a, np.float32) for a in (rotW, F, G))
    R, Wo, bo = (np.asarray(a, np.float32) for a in (R, Wo, bo))

    if "nc" not in _compiled:
        _compiled["nc"] = _build()
    nc = _compiled["nc"]

    Ball, Call, wbigS, wf, wg = _host_weights(Wi, bi, P, rotW, F, G, R, Wo)

    in_maps = []
    for c in range(NCORES):
        ts = slice(c * TLOC, (c + 1) * TLOC)
        # xtb[din, g*B + b] = x[b, c*TLOC + g, din]; ones row for the bias
        xs = x[:, ts, :]                               # [B, TLOC, DIN]
        xT = xs.transpose(2, 1, 0).reshape(DIN, NTOK)  # [DIN, g*B+b]
        xT = np.concatenate([xT, np.ones((1, NTOK), np.float32)], axis=0)
        # wpb: group g at cols g*D, tile (g,jt) = B_t[:, jt*128:(jt+1)*128]
        wpb = Ball[ts].transpose(1, 0, 2).reshape(KIN, TLOC * D)
        # wpc: tile (g,kt) at cols (g*NB+kt)*DOUT = C_t[kt*128:(kt+1)*128, :]
        wpc = Call[ts].reshape(TLOC, NB, 128, DOUT).transpose(2, 0, 1, 3) \
                      .reshape(128, TLOC * NB * DOUT)
        in_maps.append({
            "xtb": np.ascontiguousarray(xT).astype(bf16),
            "wpb": np.ascontiguousarray(wpb).astype(bf16),
            "wf": wf, "wg": wg,
            "wbigS": wbigS,
            "wpc": np.ascontiguousarray(wpc, np.float32),
        })

    from concourse.bass_utils import run_bass_kernel_spmd
    res = run_bass_kernel_spmd(nc, in_maps, list(range(NCORES)))
    _compiled["last_res"] = res

    out = np.empty((B, S, DOUT), np.float32)
    for c in range(NCORES):
        oT = res.results[c]["out"]                     # [DOUT, NTOK]
        out[:, c * TLOC:(c + 1) * TLOC, :] = \
            oT.reshape(DOUT, TLOC, B).transpose(2, 1, 0)
    out += bo.astype(np.float32)
    return out
